# revision 14
# baseline (speedup 1.0000x reference)
"""QSP expectation kernel for Trainium2 (Bass/Tile), 8-core data parallel.

Math: for the QSP sequence U = S(phi_0) * prod_{k=1..2d} [W(x) S(phi_k)] with
d=10, the output Re(U[0,0]) is exactly a degree-10 trigonometric polynomial in
theta = 2x:

    g(x) = a0 + sum_{m=1..10} A_m * sin(2m*x + ph_m)

The 21 coefficients (a0, A_1..10, ph_1..10) are recovered from the 21 phase
params by sampling the (tiny) recurrence at 64 points in float64 and taking an
FFT — exact to machine precision (residual harmonics vanish identically).

Hardware Sin (ScalarE spline) is only valid for |arg| <= ~pi, so the host
sends d = centered_mod(2x, 2pi) in [-pi, pi] instead of x (same DMA bytes),
and the device builds each harmonic angle incrementally:

    a_1 = wrap(d + ph_1),   a_m = wrap(a_{m-1} + d + dph_m),  dph pre-wrapped

where wrap is the ADD_RANGE_WRAP custom DVE op (shift, then wrap by one 2pi
into [-pi, pi]; inputs bounded by 3pi so one wrap suffices). Sin terms are
accumulated with fused scalar_tensor_tensor ops and finally scaled by alphas.
"""

import numpy as np

N = 4_000_000
NCORES = 8
PER = N // NCORES          # 500_000 elements per core
P = 128                    # SBUF partitions
FD = 3968                  # free dim per core (128*31); PER padded to P*FD
NT = 4                     # column tiles
TFD = FD // NT             # 992
DEPTH = 10
NH = 10                    # harmonics 1..10

PI = float(np.float32(np.pi))
TWO_PI = float(np.float32(2 * np.pi))

_cache = {}


def _trig_coeffs(phi):
    """Exact harmonic decomposition of the QSP expectation, in float64."""
    phi = np.asarray(phi, dtype=np.float64)
    nfft = 64
    theta = 2 * np.pi * np.arange(nfft) / nfft
    x = theta / 2
    c = np.cos(x)
    s = np.sin(x)
    a = np.exp(1j * phi[0]) * np.ones_like(x, dtype=np.complex128)
    b = np.zeros_like(a)
    for k in range(1, 2 * DEPTH + 1):
        p = np.exp(1j * phi[k])
        ta = a * c + b * (1j * s)
        tb = a * (1j * s) + b * c
        a = ta * p
        b = tb * np.conj(p)
    g = a.real  # Re(U[0,0]) on the sample grid
    F = np.fft.rfft(g) / nfft
    a0 = F[0].real
    am = 2 * F.real          # cos(m theta) coefficients
    bm = -2 * F.imag         # sin(m theta) coefficients
    A = np.hypot(am, bm)[1 : NH + 1]
    ph = np.arctan2(am, bm)[1 : NH + 1]
    return float(a0), A, ph


def _wrap_pi(v):
    """Centered mod into [-pi, pi)."""
    return np.mod(np.asarray(v, np.float64) + np.pi, 2 * np.pi) - np.pi


def _build_nc(a0, A, ph, nt=NT, gp_add=False, gp_acc=0, gp_mul=False):
    """gp_add: angle-chain tensor_adds on GpSimd; gp_acc: how many of the 9
    accumulation STTs go to GpSimd; gp_mul: final alpha-multiply on GpSimd."""
    import concourse.bacc as bacc
    import concourse.mybir as mybir
    import concourse.tile as tile

    f32 = mybir.dt.float32
    Sin = mybir.ActivationFunctionType.Sin
    mult = mybir.AluOpType.mult
    add = mybir.AluOpType.add

    tfd = FD // nt

    # Per-step phase increments, pre-wrapped so |a_prev + d + dph| <= 3pi.
    dph = _wrap_pi(np.diff(ph))

    nc = bacc.Bacc()
    xin = nc.dram_tensor("x", [P, FD], f32, kind="ExternalInput")
    alf = nc.dram_tensor("alphas", [P, FD], f32, kind="ExternalInput")
    out = nc.dram_tensor("out", [P, FD], f32, kind="ExternalOutput")

    with tile.TileContext(nc) as tc:
        with (
            tc.tile_pool(name="io", bufs=3) as io_pool,
            tc.tile_pool(name="ang", bufs=10) as ang_pool,
            tc.tile_pool(name="raw", bufs=4) as raw_pool,
            tc.tile_pool(name="dd", bufs=3) as dd_pool,
            tc.tile_pool(name="terms", bufs=6) as term_pool,
            tc.tile_pool(name="acc", bufs=6) as acc_pool,
        ):
            for t in range(nt):
                sl = slice(t * tfd, (t + 1) * tfd)
                dt = io_pool.tile([P, tfd], f32, tag="dt")
                nc.sync.dma_start(out=dt[:], in_=xin[:, sl])
                at = io_pool.tile([P, tfd], f32, tag="at")
                nc.sync.dma_start(out=at[:], in_=alf[:, sl])

                add_eng = nc.gpsimd if gp_add else nc.vector

                def wrapped(src, shift, tag="ang"):
                    pool = dd_pool if tag in ("d2", "d4") else ang_pool
                    o = pool.tile([P, tfd], f32, tag=tag)
                    nc.vector.add_range_wrap(o[:], src[:], float(shift), PI, TWO_PI)
                    return o

                def add_wrap(x1, x2, shift):
                    raw = raw_pool.tile([P, tfd], f32, tag="raw")
                    add_eng.tensor_add(raw[:], x1[:], x2[:])
                    return wrapped(raw, shift)

                # Doubled deltas: d2 = wrap(2d), d4 = wrap(2*d2). Gives four
                # parallel angle chains of depth <=3 instead of one of 10.
                d2r = dd_pool.tile([P, tfd], f32, tag="d2r")
                nc.vector.tensor_scalar(d2r[:], dt[:], 2.0, None, mult)
                d2 = wrapped(d2r, 0.0, tag="d2")
                d4r = dd_pool.tile([P, tfd], f32, tag="d4r")
                nc.vector.tensor_scalar(d4r[:], d2[:], 2.0, None, mult)
                d4 = wrapped(d4r, 0.0, tag="d4")

                a = [None] * (NH + 1)
                a[1] = wrapped(dt, ph[0])
                a[2] = wrapped(d2, ph[1])
                a[3] = add_wrap(d2, dt, ph[2])
                a[4] = wrapped(d4, ph[3])
                for m in range(5, NH + 1):
                    a[m] = add_wrap(a[m - 4], d4, _wrap_pi(ph[m - 1] - ph[m - 5]))

                terms = [None] * (NH + 1)
                for m in range(1, NH + 1):
                    term = term_pool.tile([P, tfd], f32, tag="term")
                    nc.scalar.activation(term[:], a[m][:], Sin, bias=0.0, scale=1.0)
                    terms[m] = term

                # Two accumulation half-chains, combined at the end.
                def half_acc(ms, base, n_gp):
                    acc = None
                    for i, m in enumerate(ms):
                        nacc = acc_pool.tile([P, tfd], f32, tag="acc")
                        if acc is None:
                            nc.vector.tensor_scalar(
                                nacc[:], terms[m][:], float(A[m - 1]), float(base),
                                mult, add,
                            )
                        else:
                            eng = nc.gpsimd if i <= n_gp else nc.vector
                            eng.scalar_tensor_tensor(
                                nacc[:], terms[m][:], float(A[m - 1]), acc[:],
                                mult, add,
                            )
                        acc = nacc
                    return acc

                acc_a = half_acc([1, 3, 5, 7, 9], a0, gp_acc)
                acc_b = half_acc([2, 4, 6, 8, 10], 0.0, gp_acc)
                tot = acc_pool.tile([P, tfd], f32, tag="tot")
                (nc.gpsimd if gp_mul else nc.vector).tensor_add(
                    tot[:], acc_a[:], acc_b[:]
                )
                ot = io_pool.tile([P, tfd], f32, tag="ot")
                (nc.gpsimd if gp_mul else nc.vector).tensor_mul(ot[:], tot[:], at[:])
                nc.sync.dma_start(out=out[:, sl], in_=ot[:])
    nc.finalize()
    return nc


def _get_runner(key):
    if key not in _cache:
        phi = np.frombuffer(key, dtype=np.float32)
        a0, A, ph = _trig_coeffs(phi)
        _cache[key] = _build_nc(a0, A, ph)
    return _cache[key]


def kernel(x, qsp_params, alphas):
    from concourse.bass_utils import run_bass_kernel_spmd

    x = np.asarray(x, dtype=np.float32).reshape(-1)
    alphas = np.ascontiguousarray(np.asarray(alphas, dtype=np.float32).reshape(-1))
    qsp_params = np.asarray(qsp_params, dtype=np.float32).reshape(-1)
    assert x.shape[0] == N and alphas.shape[0] == N

    nc = _get_runner(qsp_params.tobytes())

    # Host-side single range reduction: d = centered_mod(2x, 2pi).
    d = _wrap_pi(2.0 * x.astype(np.float64)).astype(np.float32)

    pad = P * FD - PER
    in_maps = []
    for c in range(NCORES):
        ds = d[c * PER : (c + 1) * PER]
        als = alphas[c * PER : (c + 1) * PER]
        in_maps.append(
            {
                "x": np.pad(ds, (0, pad)).reshape(P, FD),
                "alphas": np.pad(als, (0, pad)).reshape(P, FD),
            }
        )

    res = run_bass_kernel_spmd(nc, in_maps, core_ids=list(range(NCORES)))
    outs = [r["out"].reshape(-1)[:PER] for r in res.results]
    return np.concatenate(outs).astype(np.float32)[:, None]


# revision 17
# speedup vs baseline: 1.0469x; 1.0469x over previous
"""QSP expectation kernel for Trainium2 (Bass/Tile), 8-core data parallel.

Math: for the QSP sequence U = S(phi_0) * prod_{k=1..2d} [W(x) S(phi_k)] with
d=10, the output Re(U[0,0]) is exactly a degree-10 trigonometric polynomial in
theta = 2x:

    g(x) = a0 + sum_{m=1..10} A_m * sin(2m*x + ph_m)

The 21 coefficients (a0, A_1..10, ph_1..10) are recovered from the 21 phase
params by sampling the (tiny) recurrence at 64 points in float64 and taking an
FFT — exact to machine precision (residual harmonics vanish identically).

Hardware Sin (ScalarE spline) is only valid for |arg| <= ~pi, so the host
sends d = centered_mod(2x, 2pi) in [-pi, pi] instead of x (same DMA bytes),
and the device builds each harmonic angle incrementally:

    a_1 = wrap(d + ph_1),   a_m = wrap(a_{m-1} + d + dph_m),  dph pre-wrapped

where wrap is the ADD_RANGE_WRAP custom DVE op (shift, then wrap by one 2pi
into [-pi, pi]; inputs bounded by 3pi so one wrap suffices). Sin terms are
accumulated with fused scalar_tensor_tensor ops and finally scaled by alphas.
"""

import numpy as np

N = 4_000_000
NCORES = 8
PER = N // NCORES          # 500_000 elements per core
P = 128                    # SBUF partitions
FD = 3912                  # free dim per core; PER=500000 padded to P*FD=500736
NT = 4                     # column tiles
TFD = FD // NT             # 978
DEPTH = 10
NH = 10                    # harmonics 1..10

PI = float(np.float32(np.pi))
TWO_PI = float(np.float32(2 * np.pi))

_cache = {}


def _trig_coeffs(phi):
    """Exact harmonic decomposition of the QSP expectation, in float64."""
    phi = np.asarray(phi, dtype=np.float64)
    nfft = 64
    theta = 2 * np.pi * np.arange(nfft) / nfft
    x = theta / 2
    c = np.cos(x)
    s = np.sin(x)
    a = np.exp(1j * phi[0]) * np.ones_like(x, dtype=np.complex128)
    b = np.zeros_like(a)
    for k in range(1, 2 * DEPTH + 1):
        p = np.exp(1j * phi[k])
        ta = a * c + b * (1j * s)
        tb = a * (1j * s) + b * c
        a = ta * p
        b = tb * np.conj(p)
    g = a.real  # Re(U[0,0]) on the sample grid
    F = np.fft.rfft(g) / nfft
    a0 = F[0].real
    am = 2 * F.real          # cos(m theta) coefficients
    bm = -2 * F.imag         # sin(m theta) coefficients
    A = np.hypot(am, bm)[1 : NH + 1]
    ph = np.arctan2(am, bm)[1 : NH + 1]
    return float(a0), A, ph


def _wrap_pi(v):
    """Centered mod into [-pi, pi)."""
    return np.mod(np.asarray(v, np.float64) + np.pi, 2 * np.pi) - np.pi


def _build_nc(a0, A, ph, nt=NT, gp_add=False, gp_acc=0, gp_mul=False):
    """gp_add: angle-chain tensor_adds on GpSimd; gp_acc: how many of the 9
    accumulation STTs go to GpSimd; gp_mul: final alpha-multiply on GpSimd."""
    import concourse.bacc as bacc
    import concourse.mybir as mybir
    import concourse.tile as tile

    f32 = mybir.dt.float32
    Sin = mybir.ActivationFunctionType.Sin
    mult = mybir.AluOpType.mult
    add = mybir.AluOpType.add

    tfd = FD // nt

    # Per-step phase increments, pre-wrapped so |a_prev + d + dph| <= 3pi.
    dph = _wrap_pi(np.diff(ph))

    nc = bacc.Bacc()
    xin = nc.dram_tensor("x", [P, FD], f32, kind="ExternalInput")
    x4in = nc.dram_tensor("x4", [P, FD], f32, kind="ExternalInput")
    alf = nc.dram_tensor("alphas", [P, FD], f32, kind="ExternalInput")
    out = nc.dram_tensor("out", [P, FD], f32, kind="ExternalOutput")

    with tile.TileContext(nc) as tc:
        with (
            tc.tile_pool(name="io", bufs=3) as io_pool,
            tc.tile_pool(name="ang", bufs=10) as ang_pool,
            tc.tile_pool(name="raw", bufs=4) as raw_pool,
            tc.tile_pool(name="terms", bufs=6) as term_pool,
            tc.tile_pool(name="acc", bufs=6) as acc_pool,
            tc.tile_pool(name="tot", bufs=2) as tot_pool,
        ):
            for t in range(nt):
                sl = slice(t * tfd, (t + 1) * tfd)
                dt = io_pool.tile([P, tfd], f32, tag="dt")
                nc.sync.dma_start(out=dt[:], in_=xin[:, sl])
                at = io_pool.tile([P, tfd], f32, tag="at")
                nc.sync.dma_start(out=at[:], in_=alf[:, sl])
                d4 = io_pool.tile([P, tfd], f32, tag="d4")
                nc.sync.dma_start(out=d4[:], in_=x4in[:, sl])

                add_eng = nc.gpsimd if gp_add else nc.vector

                def wrapped(src, shift, tag="ang"):
                    o = ang_pool.tile([P, tfd], f32, tag=tag)
                    nc.vector.add_range_wrap(o[:], src[:], float(shift), PI, TWO_PI)
                    return o

                def add_wrap(x1, x2, shift):
                    raw = raw_pool.tile([P, tfd], f32, tag="raw")
                    add_eng.tensor_add(raw[:], x1[:], x2[:])
                    return wrapped(raw, shift)

                # Serial head a1..a4 (step d), then four parallel tail
                # chains stepping by host-precomputed d4 = wrap(8x).
                a = [None] * (NH + 1)
                a[1] = wrapped(dt, ph[0])
                for m in (2, 3, 4):
                    a[m] = add_wrap(a[m - 1], dt, _wrap_pi(ph[m - 1] - ph[m - 2]))
                for m in range(5, NH + 1):
                    a[m] = add_wrap(a[m - 4], d4, _wrap_pi(ph[m - 1] - ph[m - 5]))

                terms = [None] * (NH + 1)
                for m in range(1, NH + 1):
                    term = term_pool.tile([P, tfd], f32, tag="term")
                    nc.scalar.activation(term[:], a[m][:], Sin, bias=0.0, scale=1.0)
                    terms[m] = term

                # Two accumulation half-chains, combined at the end.
                def half_acc(ms, base, n_gp):
                    acc = None
                    for i, m in enumerate(ms):
                        nacc = acc_pool.tile([P, tfd], f32, tag="acc")
                        if acc is None:
                            nc.vector.tensor_scalar(
                                nacc[:], terms[m][:], float(A[m - 1]), float(base),
                                mult, add,
                            )
                        else:
                            eng = nc.gpsimd if i <= n_gp else nc.vector
                            eng.scalar_tensor_tensor(
                                nacc[:], terms[m][:], float(A[m - 1]), acc[:],
                                mult, add,
                            )
                        acc = nacc
                    return acc

                acc_a = half_acc([1, 3, 5, 7, 9], a0, gp_acc)
                acc_b = half_acc([2, 4, 6, 8, 10], 0.0, gp_acc)
                tot = tot_pool.tile([P, tfd], f32, tag="tot")
                (nc.gpsimd if gp_mul else nc.vector).tensor_add(
                    tot[:], acc_a[:], acc_b[:]
                )
                ot = io_pool.tile([P, tfd], f32, tag="ot")
                (nc.gpsimd if gp_mul else nc.vector).tensor_mul(ot[:], tot[:], at[:])
                nc.sync.dma_start(out=out[:, sl], in_=ot[:])
    nc.finalize()
    return nc


def _get_runner(key):
    if key not in _cache:
        phi = np.frombuffer(key, dtype=np.float32)
        a0, A, ph = _trig_coeffs(phi)
        _cache[key] = _build_nc(a0, A, ph)
    return _cache[key]


def kernel(x, qsp_params, alphas):
    from concourse.bass_utils import run_bass_kernel_spmd

    x = np.asarray(x, dtype=np.float32).reshape(-1)
    alphas = np.ascontiguousarray(np.asarray(alphas, dtype=np.float32).reshape(-1))
    qsp_params = np.asarray(qsp_params, dtype=np.float32).reshape(-1)
    assert x.shape[0] == N and alphas.shape[0] == N

    nc = _get_runner(qsp_params.tobytes())

    # Host-side range reductions: d = centered_mod(2x), d4 = centered_mod(8x).
    xf = x.astype(np.float64)
    d = _wrap_pi(2.0 * xf).astype(np.float32)
    d4 = _wrap_pi(8.0 * xf).astype(np.float32)

    pad = P * FD - PER
    in_maps = []
    for c in range(NCORES):
        ds = d[c * PER : (c + 1) * PER]
        d4s = d4[c * PER : (c + 1) * PER]
        als = alphas[c * PER : (c + 1) * PER]
        in_maps.append(
            {
                "x": np.pad(ds, (0, pad)).reshape(P, FD),
                "x4": np.pad(d4s, (0, pad)).reshape(P, FD),
                "alphas": np.pad(als, (0, pad)).reshape(P, FD),
            }
        )

    res = run_bass_kernel_spmd(nc, in_maps, core_ids=list(range(NCORES)))
    outs = [r["out"].reshape(-1)[:PER] for r in res.results]
    return np.concatenate(outs).astype(np.float32)[:, None]
